# revision 2
# baseline (speedup 1.0000x reference)
"""FPLPGCN (2x GCNConv feature prop + 10x label prop + fuse) on 8 trn2 cores.

Strategy (graph/data parallel):
- Nodes sorted by in-degree, striped round-robin across 8 cores.  Each core
  owns NPAD=12544 rows (44 zero "fake" rows pad 100000 -> 100352).
- GCN refactor: out[n] = dinv[n]*(sum_{e->n} u'[src_e] + u'[n]) + b with
  u' = dinv*(z @ W).  Self term is appended as a regular (n,n) edge.
- Tables are bf16 and AllGather'ed each round.  Rounds 1-2 use a paired
  table [u_feat 64 | u_label 32 | pad 32] so one gather serves both chains
  (10 gather rounds total instead of 12).
- Aggregation: dense-packed dma_gather slots (256B elems), then per
  128-slot column a one-hot (dst-partition) matrix is built on DVE
  (is_equal vs iota) and the PE matmul-accumulates messages into per-window
  PSUM.  No per-node slot padding -> ~2x fewer descriptors than the
  node-aligned layout.
"""

import sys

sys.path.insert(0, "/opt/trn_rl_repo")

import numpy as np

NC = 8
P = 128
NPAD = 12544           # local rows per core (98 windows of 128)
NWIN = NPAD // P       # 98
TABROWS = NC * NPAD    # 100352
CHP = TABROWS // 4     # 25088: paired-layout chunk (row stride 256B)
IN_DIM, HID, OUT, DW = 128, 64, 32, 64
NUM_LBL = 10
SPAN = 5               # windows per gather-call group
MAXCOLS = 32           # <=4096 idxs per dma_gather call


def _cdiv(a, b):
    return -(-a // b)


# ----------------------------------------------------------------------------
# host-side index preprocessing (pure index manipulation; no FP math on data)
# ----------------------------------------------------------------------------

def _build_layout(ecore, ewin, epart, grp, idxv):
    """Dense slot layout for one gather addressing scheme.

    Returns dict with per-core idx16 + dpart buffers and the static column
    tables used to emit the device program.
    """
    EA = ecore.shape[0]
    # counts per (core, win, grp)
    key = (ecore * NWIN + ewin) * 4 + grp
    cnt = np.bincount(key, minlength=NC * NWIN * 4).reshape(NC, NWIN, 4)
    ncols = _cdiv(cnt.max(axis=0), 128)                     # [NWIN, 4]

    spans = [(w0, min(w0 + SPAN, NWIN)) for w0 in range(0, NWIN, SPAN)]
    # global column allocation: (span, grp, win) order
    col_start = np.zeros((NWIN, 4), np.int64)
    region_col0 = np.zeros((len(spans), 4), np.int64)       # first col of region
    region_off16 = np.zeros((len(spans), 4), np.int64)      # idx16 col offset
    col = 0
    off16 = 0
    for s, (w0, w1) in enumerate(spans):
        for g in range(4):
            region_col0[s, g] = col
            region_off16[s, g] = off16
            for w in range(w0, w1):
                col_start[w, g] = col
                col += int(ncols[w, g])
            off16 += int((col - region_col0[s, g]) * 8)
    totcols = col
    tot16 = off16

    # per-edge slot position within its (core, win, grp) group
    o = np.argsort(key, kind="stable")
    ks = key[o]
    first = np.searchsorted(ks, ks, side="left")
    pos = np.empty(EA, np.int64)
    pos[o] = np.arange(EA) - first

    span_of_w = np.arange(NWIN) // SPAN
    ecolg = col_start[ewin, grp] + pos // 128
    part = pos % 128
    es = span_of_w[ewin]
    i_in_region = (ecolg - region_col0[es, grp]) * 128 + part

    idx16 = np.zeros((NC, 16, tot16), np.int16)
    idx16[ecore, i_in_region % 16,
          region_off16[es, grp] + i_in_region // 16] = idxv.astype(np.int16)

    dpart = np.full((NC, 128, totcols), 128.0, np.float32)
    dpart[ecore, part, ecolg] = epart.astype(np.float32)

    # static tables for program emission
    return dict(idx16=idx16, dpart=dpart, ncols=ncols, spans=spans,
                region_col0=region_col0, region_off16=region_off16,
                totcols=totcols, tot16=tot16)


def _preprocess(edge_index, n_nodes):
    src = np.ascontiguousarray(edge_index[0]).astype(np.int64)
    dst = np.ascontiguousarray(edge_index[1]).astype(np.int64)
    deg = np.bincount(dst, minlength=n_nodes).astype(np.int64)

    order = np.argsort(deg, kind="stable")          # ascending in-degree
    rank = np.empty(n_nodes, np.int64)
    rank[order] = np.arange(TABROWS - n_nodes, TABROWS)  # fakes get ranks 0..351
    core_of = rank % NC
    local_of = rank // NC
    trow = core_of * NPAD + local_of                # table row per real node

    # append self edges (n, n) for every real node
    s_all = np.concatenate([src, np.arange(n_nodes, dtype=np.int64)])
    d_all = np.concatenate([dst, np.arange(n_nodes, dtype=np.int64)])
    etr = trow[s_all]
    ecore = core_of[d_all]
    eloc = local_of[d_all]
    ewin = eloc // P
    epart = eloc % P

    layP = _build_layout(ecore, ewin, epart, etr // CHP, etr % CHP)
    layL = _build_layout(ecore, ewin, epart, etr % 4, etr // 4)

    meta = dict(order=order, core_of=core_of, local_of=local_of, trow=trow,
                deg=deg, layP=layP, layL=layL)
    return meta


def _shard_nodes(arr, core_of, local_of, width, dtype=np.float32):
    n = arr.shape[0]
    out = np.zeros((NC, NPAD, width), dtype)
    a2 = np.asarray(arr, dtype).reshape(n, width)
    out[core_of, local_of] = a2
    return out


# ----------------------------------------------------------------------------
# device program
# ----------------------------------------------------------------------------

def _build(meta, nonzero_b):
    import concourse.bacc as bacc
    import concourse.bass as bass
    import concourse.mybir as mybir
    import concourse.tile as tile

    f32 = mybir.dt.float32
    bf16 = mybir.dt.bfloat16
    layP, layL = meta["layP"], meta["layL"]

    TABP_ALLOC = TABROWS           # paired rows: elem = full row, no spill
    TABL_ALLOC = TABROWS + 8       # label: elem spans 4 rows -> slack

    nc = bacc.Bacc("TRN2", target_bir_lowering=False, debug=False,
                   num_devices=NC, num_swdge_queues=4)

    x_sh = nc.dram_tensor("x_sh", [NPAD, IN_DIM], f32, kind="ExternalInput")
    y_sh = nc.dram_tensor("y_sh", [NPAD, OUT], f32, kind="ExternalInput")
    dw_sh = nc.dram_tensor("dw_sh", [NPAD, DW], f32, kind="ExternalInput")
    mask_sh = nc.dram_tensor("mask_sh", [NPAD, 1], mybir.dt.int8,
                             kind="ExternalInput")
    deg_sh = nc.dram_tensor("deg_sh", [NPAD, 1], mybir.dt.int32,
                            kind="ExternalInput")
    idxP_d = nc.dram_tensor("idxP_d", [P, layP["tot16"]], mybir.dt.int16,
                            kind="ExternalInput")
    idxL_d = nc.dram_tensor("idxL_d", [P, layL["tot16"]], mybir.dt.int16,
                            kind="ExternalInput")
    dpartP_d = nc.dram_tensor("dpartP_d", [P, layP["totcols"]], bf16,
                              kind="ExternalInput")
    dpartL_d = nc.dram_tensor("dpartL_d", [P, layL["totcols"]], bf16,
                              kind="ExternalInput")
    W0_d = nc.dram_tensor("W0", [IN_DIM, HID], f32, kind="ExternalInput")
    W1_d = nc.dram_tensor("W1", [HID, HID], f32, kind="ExternalInput")
    Wl_d = nc.dram_tensor("Wl", [NUM_LBL * OUT, OUT], f32, kind="ExternalInput")
    Wf_d = nc.dram_tensor("Wf", [HID + OUT + DW, OUT], f32, kind="ExternalInput")
    b_d = nc.dram_tensor("b_all", [4, max(HID, OUT) * NUM_LBL], f32,
                         kind="ExternalInput")
    out_sh = nc.dram_tensor("out_sh", [NPAD, OUT], f32, kind="ExternalOutput")

    # internal DRAM
    tabP = [nc.dram_tensor(f"tabP{i}", [TABP_ALLOC, 2 * HID], bf16,
                           addr_space="Shared") for i in range(2)]
    tabL = [nc.dram_tensor(f"tabL{i}", [TABL_ALLOC, OUT], bf16,
                           addr_space="Shared") for i in range(2)]
    bnP = [nc.dram_tensor(f"bnP{i}", [NPAD, 2 * HID], bf16) for i in range(2)]
    bnL = [nc.dram_tensor(f"bnL{i}", [NPAD, OUT], bf16) for i in range(2)]

    with tile.TileContext(nc) as tc:
        with tc.tile_pool(name="persist", bufs=1) as pp, \
             tc.tile_pool(name="g", bufs=3) as gp, \
             tc.tile_pool(name="ix", bufs=2) as ixp, \
             tc.tile_pool(name="oh", bufs=4) as ohp, \
             tc.tile_pool(name="wk", bufs=3) as wk, \
             tc.tile_pool(name="ps", bufs=4, space="PSUM") as ps, \
             tc.tile_pool(name="psw", bufs=8, space="PSUM") as psw:

            # ---- constants / persistent state ----
            W0 = pp.tile([IN_DIM, HID], f32)
            nc.sync.dma_start(out=W0[:], in_=W0_d[:, :])
            W1 = pp.tile([P, HID], f32)
            for a in range(P // HID):
                nc.sync.dma_start(out=W1[a * HID:(a + 1) * HID, :], in_=W1_d[:, :])
            Wl = pp.tile([P, NUM_LBL * OUT], f32)
            for j in range(NUM_LBL):
                for a in range(3):  # lhsT base partitions 0/32/64
                    nc.sync.dma_start(
                        out=Wl[a * OUT:(a + 1) * OUT, j * OUT:(j + 1) * OUT],
                        in_=Wl_d[j * OUT:(j + 1) * OUT, :])
            Wfa = pp.tile([128, OUT], f32)
            nc.sync.dma_start(out=Wfa[:], in_=Wf_d[0:128, :])
            Wfb = pp.tile([HID + OUT + DW - 128, OUT], f32)
            nc.sync.dma_start(out=Wfb[:], in_=Wf_d[128:, :])
            from concourse.masks import make_identity
            ident = pp.tile([P, P], f32)
            make_identity(nc, ident[:])

            # iota along free dim, bf16 (values 0..127 exact)
            iota16 = pp.tile([1, P], mybir.dt.int16)
            nc.vector.iota(out=iota16[:], pattern=[[1, P]], base=0,
                           channel_multiplier=0)
            iotab = pp.tile([1, P], bf16)
            nc.vector.tensor_copy(out=iotab[:], in_=iota16[:])
            iota_bf = pp.tile([P, P], bf16)
            # broadcast row to all partitions via matmul with ones column
            onecol = pp.tile([1, P], f32)
            nc.vector.memset(onecol[:], 1.0)
            iotaf = pp.tile([1, P], f32)
            nc.vector.tensor_copy(out=iotaf[:], in_=iota16[:])
            iops = ps.tile([P, P], f32, tag="iops", name="iops")
            nc.tensor.matmul(out=iops[:], lhsT=onecol[:], rhs=iotaf[:],
                             start=True, stop=True)
            nc.vector.tensor_copy(out=iota_bf[:], in_=iops[:])

            # dst-partition tables (resident)
            dpartP = pp.tile([P, layP["totcols"]], bf16)
            nc.sync.dma_start(out=dpartP[:], in_=dpartP_d[:, :])
            dpartL = pp.tile([P, layL["totcols"]], bf16)
            nc.sync.dma_start(out=dpartL[:], in_=dpartL_d[:, :])

            yb = pp.tile([P, NWIN * OUT], f32)
            nc.sync.dma_start(
                out=yb[:].rearrange("p (w f) -> p w f", w=NWIN),
                in_=y_sh[:, :].rearrange("(w p) f -> p w f", p=P))
            maskb = pp.tile([P, NWIN], mybir.dt.int8)
            nc.sync.dma_start(
                out=maskb[:], in_=mask_sh[:, 0].rearrange("(w p) -> p w", p=P))
            degb = pp.tile([P, NWIN], mybir.dt.int32)
            nc.sync.dma_start(
                out=degb[:], in_=deg_sh[:, 0].rearrange("(w p) -> p w", p=P))

            degf = pp.tile([P, NWIN], f32)
            nc.vector.tensor_copy(out=degf[:], in_=degb[:])
            recipb = pp.tile([P, NWIN], f32)
            nc.vector.tensor_scalar(out=degf[:], in0=degf[:], scalar1=1.0,
                                    scalar2=None, op0=mybir.AluOpType.add)
            nc.vector.reciprocal(out=recipb[:], in_=degf[:])      # 1/(deg+1)
            dinvb = pp.tile([P, NWIN], f32)
            nc.scalar.sqrt(out=dinvb[:], in_=recipb[:])           # 1/sqrt(deg+1)
            # zero fake lanes (window 0, partitions 0..43)
            nfake = TABROWS - 100000
            nc.vector.memset(recipb[0:nfake // NC, 0:1], 0.0)
            nc.vector.memset(dinvb[0:nfake // NC, 0:1], 0.0)
            dinvy = pp.tile([P, NWIN * OUT], f32)
            for w in range(NWIN):
                nc.vector.tensor_scalar(
                    out=dinvy[:, w * OUT:(w + 1) * OUT],
                    in0=yb[:, w * OUT:(w + 1) * OUT],
                    scalar1=dinvb[:, w:w + 1], scalar2=None,
                    op0=mybir.AluOpType.mult)

            def bias_tile(row, width):
                bt = pp.tile([P, width], f32, tag=f"bias{row}", name=f"bias{row}")
                brow = pp.tile([1, width], f32, tag=f"brow{row}", name=f"brow{row}")
                nc.sync.dma_start(out=brow[:], in_=b_d[row:row + 1, 0:width])
                pt = ps.tile([P, width], f32, tag="biasps", name="biasps")
                nc.tensor.matmul(out=pt[:], lhsT=onecol[:], rhs=brow[:],
                                 start=True, stop=True)
                nc.vector.tensor_copy(out=bt[:], in_=pt[:])
                return bt

            bias0 = bias_tile(0, HID) if nonzero_b[0] else None
            bias1 = bias_tile(1, HID) if nonzero_b[1] else None
            biasf = bias_tile(3, OUT) if nonzero_b[3] else None

            vF = pp.tile([P, NWIN * HID], f32)      # next-round feature input
            vL = pp.tile([P, NWIN * OUT], f32)      # next-round label input
            hfin = pp.tile([P, NWIN * HID], f32)
            xlfin = pp.tile([P, NWIN * OUT], f32)
            ubP = pp.tile([P, NWIN * 2 * HID], bf16)  # paired bounce staging
            ubL = pp.tile([P, NWIN * OUT], bf16)      # label bounce staging
            nc.vector.memset(ubP[:], 0.0)             # zero pad lanes once

            # ---- aggregation engine ----
            def stage_agg(lay, tab_in_aps, dpart_t, idx_d, F, out_cb):
                """Gather + one-hot matmul segment-sum; out_cb(w, psum_ap)."""
                ncols = lay["ncols"]
                spans = lay["spans"]
                region_col0 = lay["region_col0"]
                region_off16 = lay["region_off16"]
                qctr = [0]
                for s, (w0, w1) in enumerate(spans):
                    # idx tile for the whole span
                    span_off16 = int(region_off16[s, 0])
                    span_cols = int(sum(ncols[w, g] for w in range(w0, w1)
                                        for g in range(4)))
                    if span_cols == 0:
                        continue
                    span_n16 = span_cols * 8
                    ixt = ixp.tile([P, span_n16], mybir.dt.int16, tag="ix")
                    nc.sync.dma_start(
                        out=ixt[:], in_=idx_d[:, span_off16:span_off16 + span_n16])

                    gts = {}
                    for g in range(4):
                        rcols = int(sum(ncols[w, g] for w in range(w0, w1)))
                        if rcols == 0:
                            continue
                        gt = gp.tile([P, rcols * P], bf16, tag=f"g{g}")
                        gts[g] = gt
                        o16 = int(region_off16[s, g]) - span_off16
                        for c0 in range(0, rcols, MAXCOLS):
                            c1 = min(c0 + MAXCOLS, rcols)
                            nidx = (c1 - c0) * P
                            nc.gpsimd.dma_gather(
                                out_ap=gt[:, c0 * P:c1 * P]
                                    .rearrange("p (s f) -> p s f", f=P),
                                in_ap=tab_in_aps[g],
                                idxs_ap=ixt[:, o16 + c0 * 8:o16 + c1 * 8],
                                num_idxs=nidx, num_idxs_reg=nidx,
                                elem_size=P, queue_num=qctr[0] % 4,
                                single_packet=False)
                            qctr[0] += 1
                    # per-window one-hot matmul accumulation
                    for w in range(w0, w1):
                        wcols = [(g, int(ncols[w, g])) for g in range(4)
                                 if ncols[w, g] > 0]
                        if not wcols:
                            continue
                        tot = sum(n for _, n in wcols)
                        pt = psw.tile([P, F], f32, tag="aggps")
                        k = 0
                        for g, n in wcols:
                            gbase = int(col_off(lay, s, g, w))
                            for c in range(n):
                                oh = ohp.tile([P, P], bf16, tag="oh")
                                colg = int(lay_colstart(lay, w, g)) + c
                                nc.vector.tensor_scalar(
                                    out=oh[:], in0=iota_bf[:],
                                    scalar1=dpart_t[:, colg:colg + 1],
                                    scalar2=None,
                                    op0=mybir.AluOpType.is_equal)
                                nc.tensor.matmul(
                                    out=pt[:], lhsT=oh[:],
                                    rhs=gts[g][:, (gbase + c) * P:
                                               (gbase + c) * P + F],
                                    start=(k == 0), stop=(k == tot - 1))
                                k += 1
                        out_cb(w, pt)

            def lay_colstart(lay, w, g):
                # global col index of first col of (w, g)
                s = w // SPAN
                c = lay["region_col0"][s, g]
                for w2 in range(s * SPAN, w):
                    c += lay["ncols"][w2, g]
                return c

            def col_off(lay, s, g, w):
                # col offset of window w inside region (s, g)'s gather tile
                c = 0
                for w2 in range(s * SPAN, w):
                    c += lay["ncols"][w2, g]
                return c

            # table input APs per group
            def tabP_aps(t):
                return [t[q * CHP:(q + 1) * CHP, :].rearrange(
                    "r f -> r f") for q in range(4)]

            def tabL_aps(t):
                return [t[q:q + 4 * (CHP), :].rearrange(
                    "(r k) f -> r (k f)", k=4) for q in range(4)]

            # ---- stage matmuls: u' = v @ W  (writes into ub staging, bf16) ----
            def stage_matmul(vtile, F_in, W_ap, F_out, ub, ub_off, ub_stride):
                per = min(P // F_in, 3)
                for wb in range(0, NWIN, per):
                    nwt = min(per, NWIN - wb)
                    tp = ps.tile([P, P], f32, tag="tps")
                    nc.tensor.transpose(
                        out=tp[0:nwt * F_in, :],
                        in_=vtile[:, wb * F_in:(wb + nwt) * F_in],
                        identity=ident[:])
                    vT = wk.tile([P, P], f32, tag="vT")
                    nc.scalar.copy(out=vT[0:nwt * F_in, :], in_=tp[0:nwt * F_in, :])
                    for a in range(nwt):
                        w = wb + a
                        up = ps.tile([P, F_out], f32, tag="ups")
                        nc.tensor.matmul(out=up[:],
                                         lhsT=vT[a * F_in:(a + 1) * F_in, :],
                                         rhs=W_ap[a * F_in:(a + 1) * F_in, :],
                                         start=True, stop=True)
                        nc.scalar.copy(
                            out=ub[:, w * ub_stride + ub_off:
                                   w * ub_stride + ub_off + F_out],
                            in_=up[:])

            def flush_bounce(ub, bn, width):
                nc.sync.dma_start(
                    out=bn[:, :].rearrange("(w p) f -> p w f", p=P),
                    in_=ub[:].rearrange("p (w f) -> p w f", w=NWIN))

            def allgather(bn, tab):
                nc.gpsimd.collective_compute(
                    "AllGather", bass.mybir.AluOpType.bypass,
                    replica_groups=[list(range(NC))],
                    ins=[bn[:, :].opt()],
                    outs=[tab[0:TABROWS, :].opt()])

            # ---- initial tables: uf1 = (dinv*x)@W0, ul1 = dinvy@Wl0 ----
            for w in range(NWIN):
                xt = wk.tile([P, IN_DIM], f32, tag="xt")
                nc.sync.dma_start(out=xt[:], in_=x_sh[w * P:(w + 1) * P, :])
                nc.vector.tensor_scalar(out=xt[:], in0=xt[:],
                                        scalar1=dinvb[:, w:w + 1],
                                        scalar2=None, op0=mybir.AluOpType.mult)
                tp = ps.tile([P, P], f32, tag="tps")
                nc.tensor.transpose(out=tp[:], in_=xt[:], identity=ident[:])
                vT = wk.tile([P, P], f32, tag="vT")
                nc.scalar.copy(out=vT[:], in_=tp[:])
                up = ps.tile([P, HID], f32, tag="ups")
                nc.tensor.matmul(out=up[:], lhsT=vT[:], rhs=W0[:], start=True,
                                 stop=True)
                nc.scalar.copy(out=ubP[:, w * 2 * HID:w * 2 * HID + HID],
                               in_=up[:])
            stage_matmul(dinvy, OUT, Wl[:, 0:OUT], OUT, ubP, HID, 2 * HID)
            flush_bounce(ubP, bnP[0], 2 * HID)
            allgather(bnP[0], tabP[0])

            # ---- round r0 (paired: feature conv1 + label conv1) ----
            def cb_r0(w, pt):
                dstF = vF[:, w * HID:(w + 1) * HID]
                nc.vector.tensor_scalar(out=dstF, in0=pt[:, 0:HID],
                                        scalar1=recipb[:, w:w + 1], scalar2=None,
                                        op0=mybir.AluOpType.mult)
                if bias0 is not None:
                    dv = wk.tile([P, HID], f32, tag="dbv")
                    nc.vector.tensor_scalar(out=dv[:], in0=bias0[:],
                                            scalar1=dinvb[:, w:w + 1],
                                            scalar2=None,
                                            op0=mybir.AluOpType.mult)
                    nc.vector.tensor_add(out=dstF, in0=dstF, in1=dv[:])
                dstL = vL[:, w * OUT:(w + 1) * OUT]
                nc.vector.tensor_scalar(out=dstL, in0=pt[:, HID:HID + OUT],
                                        scalar1=recipb[:, w:w + 1], scalar2=None,
                                        op0=mybir.AluOpType.mult)
                nc.vector.copy_predicated(
                    out=dstL, mask=maskb[:, w:w + 1].to_broadcast([P, OUT]),
                    data=dinvy[:, w * OUT:(w + 1) * OUT])
            stage_agg(layP, tabP_aps(tabP[0]), dpartP, idxP_d, HID + OUT, cb_r0)

            # tables for r1: uf2 = vF@W1, ul2 = vL@Wl1
            stage_matmul(vF, HID, W1[:, :], HID, ubP, 0, 2 * HID)
            stage_matmul(vL, OUT, Wl[:, OUT:2 * OUT], OUT, ubP, HID, 2 * HID)
            flush_bounce(ubP, bnP[1], 2 * HID)
            allgather(bnP[1], tabP[1])

            # ---- round r1 (paired: feature conv2 final + label conv2) ----
            def cb_r1(w, pt):
                dstF = hfin[:, w * HID:(w + 1) * HID]
                nc.vector.tensor_scalar(out=dstF, in0=pt[:, 0:HID],
                                        scalar1=dinvb[:, w:w + 1], scalar2=None,
                                        op0=mybir.AluOpType.mult)
                if bias1 is not None:
                    nc.vector.tensor_add(out=dstF, in0=dstF, in1=bias1[:])
                dstL = vL[:, w * OUT:(w + 1) * OUT]
                nc.vector.tensor_scalar(out=dstL, in0=pt[:, HID:HID + OUT],
                                        scalar1=recipb[:, w:w + 1], scalar2=None,
                                        op0=mybir.AluOpType.mult)
                nc.vector.copy_predicated(
                    out=dstL, mask=maskb[:, w:w + 1].to_broadcast([P, OUT]),
                    data=dinvy[:, w * OUT:(w + 1) * OUT])
            stage_agg(layP, tabP_aps(tabP[1]), dpartP, idxP_d, HID + OUT, cb_r1)

            # ---- label-only rounds: convs 3..10 ----
            # table for conv j (1-indexed) is ul_j = vL @ Wl[j-1]
            stage_matmul(vL, OUT, Wl[:, 2 * OUT:3 * OUT], OUT, ubL, 0, OUT)
            flush_bounce(ubL, bnL[0], OUT)
            allgather(bnL[0], tabL[0])

            for j in range(3, NUM_LBL + 1):
                last = (j == NUM_LBL)
                ti = (j - 3) % 2

                def cb_lbl(w, pt, last=last):
                    if last:
                        dst = xlfin[:, w * OUT:(w + 1) * OUT]
                        nc.vector.tensor_scalar(out=dst, in0=pt[:, 0:OUT],
                                                scalar1=dinvb[:, w:w + 1],
                                                scalar2=None,
                                                op0=mybir.AluOpType.mult)
                        nc.vector.copy_predicated(
                            out=dst,
                            mask=maskb[:, w:w + 1].to_broadcast([P, OUT]),
                            data=yb[:, w * OUT:(w + 1) * OUT])
                    else:
                        dst = vL[:, w * OUT:(w + 1) * OUT]
                        nc.vector.tensor_scalar(out=dst, in0=pt[:, 0:OUT],
                                                scalar1=recipb[:, w:w + 1],
                                                scalar2=None,
                                                op0=mybir.AluOpType.mult)
                        nc.vector.copy_predicated(
                            out=dst,
                            mask=maskb[:, w:w + 1].to_broadcast([P, OUT]),
                            data=dinvy[:, w * OUT:(w + 1) * OUT])

                stage_agg(layL, tabL_aps(tabL[ti]), dpartL, idxL_d, OUT, cb_lbl)
                if not last:
                    stage_matmul(vL, OUT, Wl[:, j * OUT:(j + 1) * OUT], OUT,
                                 ubL, 0, OUT)
                    flush_bounce(ubL, bnL[(ti + 1) % 2], OUT)
                    allgather(bnL[(ti + 1) % 2], tabL[(ti + 1) % 2])

            # ---- fuse: sigmoid([h | xl | dw] @ Wf + bf) ----
            ofin = pp.tile([P, NWIN * OUT], f32)
            for w in range(NWIN):
                dwt = wk.tile([P, DW], f32, tag="dwt")
                nc.sync.dma_start(out=dwt[:], in_=dw_sh[w * P:(w + 1) * P, :])
                fTa = wk.tile([P, P], f32, tag="fTa")
                fTb = wk.tile([DW - 32, P], f32, tag="fTb")
                tp = ps.tile([P, P], f32, tag="tps")
                nc.tensor.transpose(out=tp[0:HID, :],
                                    in_=hfin[:, w * HID:(w + 1) * HID],
                                    identity=ident[:])
                nc.scalar.copy(out=fTa[0:HID, :], in_=tp[0:HID, :])
                tp2 = ps.tile([P, P], f32, tag="tps")
                nc.tensor.transpose(out=tp2[0:OUT, :],
                                    in_=xlfin[:, w * OUT:(w + 1) * OUT],
                                    identity=ident[:])
                nc.scalar.copy(out=fTa[HID:HID + OUT, :], in_=tp2[0:OUT, :])
                tp3 = ps.tile([P, P], f32, tag="tps")
                nc.tensor.transpose(out=tp3[0:DW, :], in_=dwt[:],
                                    identity=ident[:])
                nc.scalar.copy(out=fTa[HID + OUT:P, :],
                               in_=tp3[0:P - HID - OUT, :])
                nc.scalar.copy(out=fTb[:, :], in_=tp3[P - HID - OUT:DW, :])
                op = ps.tile([P, OUT], f32, tag="ops")
                nc.tensor.matmul(out=op[:], lhsT=fTa[:], rhs=Wfa[:],
                                 start=True, stop=False)
                nc.tensor.matmul(out=op[:], lhsT=fTb[:], rhs=Wfb[:],
                                 start=False, stop=True)
                if biasf is not None:
                    nc.vector.tensor_add(out=op[:], in0=op[:], in1=biasf[:])
                nc.scalar.activation(out=ofin[:, w * OUT:(w + 1) * OUT],
                                     in_=op[:],
                                     func=bass.mybir.ActivationFunctionType.Sigmoid)
            nc.sync.dma_start(
                out=out_sh[:, :].rearrange("(w p) f -> p w f", p=P),
                in_=ofin[:].rearrange("p (w f) -> p w f", w=NWIN))

    nc.compile()
    return nc


_CACHE = {}


def kernel(x, y, edge_index, deep_walk_emb, label_input_mask,
           W_gcn0, b_gcn0, W_gcn1, b_gcn1, W_label, b_label, W_fuse, b_fuse):
    import concourse.bass_utils as bass_utils
    import ml_dtypes

    n_nodes = x.shape[0]
    ei = np.asarray(edge_index, dtype=np.int64)
    meta = _preprocess(ei, n_nodes)
    core_of, local_of = meta["core_of"], meta["local_of"]
    layP, layL = meta["layP"], meta["layL"]

    nonzero_b = (bool(np.any(np.asarray(b_gcn0))),
                 bool(np.any(np.asarray(b_gcn1))),
                 bool(np.any(np.asarray(b_label))),
                 bool(np.any(np.asarray(b_fuse))))
    if nonzero_b[2]:
        raise NotImplementedError("nonzero label bias not wired")

    key = ("k2", n_nodes, ei.shape[1], nonzero_b,
           layP["totcols"], layL["totcols"])
    if key not in _CACHE:
        _CACHE[key] = _build(meta, nonzero_b)
    nc = _CACHE[key]

    x_s = _shard_nodes(x, core_of, local_of, IN_DIM)
    y_s = _shard_nodes(y, core_of, local_of, OUT)
    dw_s = _shard_nodes(deep_walk_emb, core_of, local_of, DW)
    mk_s = _shard_nodes(np.asarray(label_input_mask, np.int8)[:, None],
                        core_of, local_of, 1, dtype=np.int8)
    dg_s = np.zeros((NC, NPAD, 1), np.int32)
    dg_s[core_of, local_of, 0] = meta["deg"].astype(np.int32)

    bmax = max(HID, OUT) * NUM_LBL
    b_all = np.zeros((4, bmax), np.float32)
    b_all[0, :HID] = np.asarray(b_gcn0, np.float32)
    b_all[1, :HID] = np.asarray(b_gcn1, np.float32)
    b_all[2, :OUT * NUM_LBL] = np.asarray(b_label, np.float32).reshape(-1)
    b_all[3, :OUT] = np.asarray(b_fuse, np.float32)

    Wl_flat = np.asarray(W_label, np.float32).reshape(NUM_LBL * OUT, OUT)
    idxP128 = np.tile(layP["idx16"], (1, 8, 1))
    idxL128 = np.tile(layL["idx16"], (1, 8, 1))
    dpP = layP["dpart"].astype(ml_dtypes.bfloat16)
    dpL = layL["dpart"].astype(ml_dtypes.bfloat16)

    in_maps = []
    for c in range(NC):
        in_maps.append({
            "x_sh": x_s[c], "y_sh": y_s[c], "dw_sh": dw_s[c],
            "mask_sh": mk_s[c], "deg_sh": dg_s[c],
            "idxP_d": idxP128[c], "idxL_d": idxL128[c],
            "dpartP_d": dpP[c], "dpartL_d": dpL[c],
            "W0": np.asarray(W_gcn0, np.float32),
            "W1": np.asarray(W_gcn1, np.float32),
            "Wl": Wl_flat,
            "Wf": np.asarray(W_fuse, np.float32),
            "b_all": b_all,
        })
    res = bass_utils.run_bass_kernel_spmd(nc, in_maps, core_ids=list(range(NC)))
    out = np.empty((n_nodes, OUT), np.float32)
    for c in range(NC):
        sel = core_of == np.int64(c)
        out[sel] = res.results[c]["out_sh"][local_of[sel]]
    return out


# revision 15
# speedup vs baseline: 1.8477x; 1.8477x over previous
"""FPLPGCN (2x GCNConv feature prop + 10x label prop + fuse) on 8 trn2 cores.

Strategy (graph/data parallel):
- Nodes sorted by in-degree, striped round-robin across 8 cores.  Each core
  owns NPAD=12544 rows (44 zero "fake" rows pad 100000 -> 100352).
- GCN refactor: out[n] = dinv[n]*(sum_{e->n} u'[src_e] + u'[n]) + b with
  u' = dinv*(z @ W).  Self term is appended as a regular (n,n) edge.
- Tables are bf16 and AllGather'ed each round.  Rounds 1-2 use a paired
  table [u_feat 64 | u_label 32 | pad 32] so one gather serves both chains
  (10 gather rounds total instead of 12).
- Aggregation: dense-packed dma_gather slots (256B elems), then per
  128-slot column a one-hot (dst-partition) matrix is built on DVE
  (is_equal vs iota) and the PE matmul-accumulates messages into per-window
  PSUM.  No per-node slot padding -> ~2x fewer descriptors than the
  node-aligned layout.
"""

import sys

sys.path.insert(0, "/opt/trn_rl_repo")

import numpy as np

NC = 8
P = 128
NPAD = 12544           # local rows per core (98 windows of 128)
NWIN = NPAD // P       # 98
TABROWS = NC * NPAD    # 100352
CHP = TABROWS // 4     # 25088: paired-layout chunk (row stride 256B)
IN_DIM, HID, OUT, DW = 128, 64, 32, 64
NUM_LBL = 10
SPAN = 3               # windows per gather-call group
MAXCOLS = 32           # <=4096 idxs per dma_gather call


def _cdiv(a, b):
    return -(-a // b)


# ----------------------------------------------------------------------------
# host-side index preprocessing (pure index manipulation; no FP math on data)
# ----------------------------------------------------------------------------

def _build_layout(ecore, ewin, epart, grp, idxv):
    """Dense slot layout for one gather addressing scheme.

    Returns dict with per-core idx16 + dpart buffers and the static column
    tables used to emit the device program.
    """
    EA = ecore.shape[0]
    # counts per (core, win, grp)
    key = (ecore * NWIN + ewin) * 4 + grp
    cnt = np.bincount(key, minlength=NC * NWIN * 4).reshape(NC, NWIN, 4)
    ncols = _cdiv(cnt.max(axis=0), 128)                     # [NWIN, 4]

    spans = [(w0, min(w0 + SPAN, NWIN)) for w0 in range(0, NWIN, SPAN)]
    # global column allocation: (span, grp, win) order
    col_start = np.zeros((NWIN, 4), np.int64)
    region_col0 = np.zeros((len(spans), 4), np.int64)       # first col of region
    region_off16 = np.zeros((len(spans), 4), np.int64)      # idx16 col offset
    col = 0
    off16 = 0
    for s, (w0, w1) in enumerate(spans):
        for g in range(4):
            region_col0[s, g] = col
            region_off16[s, g] = off16
            for w in range(w0, w1):
                col_start[w, g] = col
                col += int(ncols[w, g])
            off16 += int((col - region_col0[s, g]) * 8)
    totcols = col
    tot16 = off16

    # per-edge slot position within its (core, win, grp) group
    o = np.argsort(key, kind="stable")
    ks = key[o]
    first = np.searchsorted(ks, ks, side="left")
    pos = np.empty(EA, np.int64)
    pos[o] = np.arange(EA) - first

    span_of_w = np.arange(NWIN) // SPAN
    ecolg = col_start[ewin, grp] + pos // 128
    part = pos % 128
    es = span_of_w[ewin]
    i_in_region = (ecolg - region_col0[es, grp]) * 128 + part

    idx16 = np.zeros((NC, 16, tot16), np.int16)
    idx16[ecore, i_in_region % 16,
          region_off16[es, grp] + i_in_region // 16] = idxv.astype(np.int16)

    dpart = np.full((NC, 128, totcols), 128.0, np.float32)
    dpart[ecore, part, ecolg] = epart.astype(np.float32)

    # static tables for program emission
    return dict(idx16=idx16, dpart=dpart, ncols=ncols, spans=spans,
                region_col0=region_col0, region_off16=region_off16,
                totcols=totcols, tot16=tot16)


def _preprocess(edge_index, n_nodes):
    src = np.ascontiguousarray(edge_index[0]).astype(np.int64)
    dst = np.ascontiguousarray(edge_index[1]).astype(np.int64)
    deg = np.bincount(dst, minlength=n_nodes).astype(np.int64)

    order = np.argsort(deg, kind="stable")          # ascending in-degree
    rank = np.empty(n_nodes, np.int64)
    rank[order] = np.arange(TABROWS - n_nodes, TABROWS)  # fakes get ranks 0..351
    core_of = rank % NC
    local_of = rank // NC
    trow = core_of * NPAD + local_of                # table row per real node

    # append self edges (n, n) for every real node
    s_all = np.concatenate([src, np.arange(n_nodes, dtype=np.int64)])
    d_all = np.concatenate([dst, np.arange(n_nodes, dtype=np.int64)])
    etr = trow[s_all]
    ecore = core_of[d_all]
    eloc = local_of[d_all]
    ewin = eloc // P
    epart = eloc % P

    layP = _build_layout(ecore, ewin, epart, etr // CHP, etr % CHP)
    layL = _build_layout(ecore, ewin, epart, etr % 4, etr // 4)

    meta = dict(order=order, core_of=core_of, local_of=local_of, trow=trow,
                deg=deg, layP=layP, layL=layL)
    return meta


def _shard_nodes(arr, core_of, local_of, width, dtype=np.float32):
    n = arr.shape[0]
    out = np.zeros((NC, NPAD, width), dtype)
    a2 = np.asarray(arr, dtype).reshape(n, width)
    out[core_of, local_of] = a2
    return out


# ----------------------------------------------------------------------------
# device program
# ----------------------------------------------------------------------------

def _build(meta, nonzero_b):
    import concourse.bacc as bacc
    import concourse.bass as bass
    import concourse.mybir as mybir
    import concourse.tile as tile

    f32 = mybir.dt.float32
    bf16 = mybir.dt.bfloat16
    layP, layL = meta["layP"], meta["layL"]

    TABP_ALLOC = TABROWS           # paired rows: elem = full row, no spill
    TABL_ALLOC = TABROWS + 8       # label: elem spans 4 rows -> slack

    nc = bacc.Bacc("TRN2", target_bir_lowering=False, debug=False,
                   num_devices=NC, num_swdge_queues=4)

    x_sh = nc.dram_tensor("x_sh", [NPAD, IN_DIM], f32, kind="ExternalInput")
    y_sh = nc.dram_tensor("y_sh", [NPAD, OUT], f32, kind="ExternalInput")
    dw_sh = nc.dram_tensor("dw_sh", [NPAD, DW], f32, kind="ExternalInput")
    mask_sh = nc.dram_tensor("mask_sh", [NPAD, 1], mybir.dt.int8,
                             kind="ExternalInput")
    deg_sh = nc.dram_tensor("deg_sh", [NPAD, 1], mybir.dt.int32,
                            kind="ExternalInput")
    idxP_d = nc.dram_tensor("idxP_d", [P, layP["tot16"]], mybir.dt.int16,
                            kind="ExternalInput")
    idxL_d = nc.dram_tensor("idxL_d", [P, layL["tot16"]], mybir.dt.int16,
                            kind="ExternalInput")
    dpartP_d = nc.dram_tensor("dpartP_d", [P, layP["totcols"]], f32,
                              kind="ExternalInput")
    dpartL_d = nc.dram_tensor("dpartL_d", [P, layL["totcols"]], f32,
                              kind="ExternalInput")
    W0_d = nc.dram_tensor("W0", [IN_DIM, HID], f32, kind="ExternalInput")
    W1_d = nc.dram_tensor("W1", [HID, HID], f32, kind="ExternalInput")
    Wl_d = nc.dram_tensor("Wl", [NUM_LBL * OUT, OUT], f32, kind="ExternalInput")
    Wf_d = nc.dram_tensor("Wf", [HID + OUT + DW, OUT], f32, kind="ExternalInput")
    b_d = nc.dram_tensor("b_all", [4, max(HID, OUT) * NUM_LBL], f32,
                         kind="ExternalInput")
    out_sh = nc.dram_tensor("out_sh", [NPAD, OUT], f32, kind="ExternalOutput")

    # internal DRAM
    tabP = [nc.dram_tensor(f"tabP{i}", [TABP_ALLOC, 2 * HID], bf16,
                           addr_space="Shared") for i in range(2)]
    tabL = [nc.dram_tensor(f"tabL{i}", [TABL_ALLOC, OUT], bf16,
                           addr_space="Shared") for i in range(2)]
    bnP = [nc.dram_tensor(f"bnP{i}", [NPAD, 2 * HID], bf16) for i in range(2)]
    bnL = [nc.dram_tensor(f"bnL{i}", [NPAD, OUT], bf16) for i in range(2)]

    with tile.TileContext(nc) as tc:
        with tc.tile_pool(name="persist", bufs=1) as pp, \
             tc.tile_pool(name="g", bufs=3) as gp, \
             tc.tile_pool(name="ix", bufs=2) as ixp, \
             tc.tile_pool(name="oh", bufs=4) as ohp, \
             tc.tile_pool(name="wk", bufs=3) as wk, \
             tc.tile_pool(name="ps", bufs=2, space="PSUM") as ps, \
             tc.tile_pool(name="psu", bufs=2, space="PSUM") as psu, \
             tc.tile_pool(name="psw", bufs=4, space="PSUM") as psw:

            # ---- constants / persistent state ----
            W0 = pp.tile([IN_DIM, HID], f32)
            nc.sync.dma_start(out=W0[:], in_=W0_d[:, :])
            W1 = pp.tile([P, HID], f32)
            for a in range(P // HID):
                nc.sync.dma_start(out=W1[a * HID:(a + 1) * HID, :], in_=W1_d[:, :])
            Wl = pp.tile([P, NUM_LBL * OUT], f32)
            for j in range(NUM_LBL):
                for a in range(3):  # lhsT base partitions 0/32/64
                    nc.sync.dma_start(
                        out=Wl[a * OUT:(a + 1) * OUT, j * OUT:(j + 1) * OUT],
                        in_=Wl_d[j * OUT:(j + 1) * OUT, :])
            Wfa = pp.tile([128, OUT], f32)
            nc.sync.dma_start(out=Wfa[:], in_=Wf_d[0:128, :])
            Wfb = pp.tile([HID + OUT + DW - 128, OUT], f32)
            nc.sync.dma_start(out=Wfb[:], in_=Wf_d[128:, :])
            from concourse.masks import make_identity
            ident = pp.tile([P, P], f32)
            make_identity(nc, ident[:])

            # iota along free dim, bf16 (values 0..127 exact)
            iota16 = pp.tile([1, P], mybir.dt.int16)
            nc.gpsimd.iota(out=iota16[:], pattern=[[1, P]], base=0,
                           channel_multiplier=0)
            iota_bf = pp.tile([P, P], bf16)
            # broadcast row to all partitions via matmul with ones column
            onecol = pp.tile([1, P], f32)
            nc.vector.memset(onecol[:], 1.0)
            iotaf = pp.tile([1, P], f32)
            nc.vector.tensor_copy(out=iotaf[:], in_=iota16[:])
            iops = ps.tile([P, P], f32, tag="tps", name="iops")
            nc.tensor.matmul(out=iops[:], lhsT=onecol[:], rhs=iotaf[:],
                             start=True, stop=True)
            nc.vector.tensor_copy(out=iota_bf[:], in_=iops[:])

            # dst-partition table: one shared resident tile (paired layout is
            # only live for rounds 0-1; reloaded with the label layout after)
            dpart = pp.tile([P, max(layP["totcols"], layL["totcols"])], f32)
            nc.sync.dma_start(out=dpart[:, 0:layP["totcols"]],
                              in_=dpartP_d[:, :])

            yb = pp.tile([P, NWIN * OUT], f32)
            nc.sync.dma_start(
                out=yb[:].rearrange("p (w f) -> p w f", w=NWIN),
                in_=y_sh[:, :].rearrange("(w p) f -> p w f", p=P))
            maskb = pp.tile([P, NWIN], mybir.dt.int8)
            nc.sync.dma_start(
                out=maskb[:], in_=mask_sh[:, 0].rearrange("(w p) -> p w", p=P))
            degb = pp.tile([P, NWIN], mybir.dt.int32)
            nc.sync.dma_start(
                out=degb[:], in_=deg_sh[:, 0].rearrange("(w p) -> p w", p=P))

            degf = pp.tile([P, NWIN], f32)
            nc.vector.tensor_copy(out=degf[:], in_=degb[:])
            recipb = pp.tile([P, NWIN], f32)
            nc.vector.tensor_scalar(out=degf[:], in0=degf[:], scalar1=1.0,
                                    scalar2=None, op0=mybir.AluOpType.add)
            nc.vector.reciprocal(out=recipb[:], in_=degf[:])      # 1/(deg+1)
            dinvb = pp.tile([P, NWIN], f32)
            nc.scalar.sqrt(out=dinvb[:], in_=recipb[:])           # 1/sqrt(deg+1)
            # zero fake lanes (window 0, partitions 0..43)
            nfake = TABROWS - 100000
            nc.vector.memset(recipb[0:nfake // NC, 0:1], 0.0)
            nc.vector.memset(dinvb[0:nfake // NC, 0:1], 0.0)
            dinvy = pp.tile([P, NWIN * OUT], f32)
            for w in range(NWIN):
                nc.vector.tensor_scalar(
                    out=dinvy[:, w * OUT:(w + 1) * OUT],
                    in0=yb[:, w * OUT:(w + 1) * OUT],
                    scalar1=dinvb[:, w:w + 1], scalar2=None,
                    op0=mybir.AluOpType.mult)

            def bias_tile(row, width):
                bt = pp.tile([P, width], f32, tag=f"bias{row}", name=f"bias{row}")
                brow = pp.tile([1, width], f32, tag=f"brow{row}", name=f"brow{row}")
                nc.sync.dma_start(out=brow[:], in_=b_d[row:row + 1, 0:width])
                pt = ps.tile([P, P], f32, tag="tps", name="biasps")
                nc.tensor.matmul(out=pt[:, 0:width], lhsT=onecol[:],
                                 rhs=brow[:], start=True, stop=True)
                nc.vector.tensor_copy(out=bt[:], in_=pt[:, 0:width])
                return bt

            bias0 = bias_tile(0, HID) if nonzero_b[0] else None
            bias1 = bias_tile(1, HID) if nonzero_b[1] else None
            biasf = bias_tile(3, OUT) if nonzero_b[3] else None

            vF = pp.tile([P, NWIN * HID], f32)      # next-round feature input
            vL = pp.tile([P, NWIN * OUT], f32)      # next-round label input
            hfin = pp.tile([P, NWIN * HID], f32)
            xlfin = pp.tile([P, NWIN * OUT], f32)

            # ---- aggregation engine ----
            CALLCOLS = 16   # 2048 idxs per dma_gather call

            def stage_agg(lay, tab_in_aps, idx_d, F, out_cb):
                """Gather + one-hot matmul segment-sum; out_cb(w, psum_ap)."""
                ncols = lay["ncols"]
                spans = lay["spans"]
                region_col0 = lay["region_col0"]
                region_off16 = lay["region_off16"]
                qctr = [0]
                for s, (w0, w1) in enumerate(spans):
                    span_off16 = int(region_off16[s, 0])
                    span_cols = int(ncols[w0:w1, :].sum())
                    if span_cols == 0:
                        continue
                    span_n16 = span_cols * 8
                    ixt = ixp.tile([P, span_n16], mybir.dt.int16, tag="ix")
                    nc.sync.dma_start(
                        out=ixt[:], in_=idx_d[:, span_off16:span_off16 + span_n16])

                    # per-window matmul chain bookkeeping: k-th matmul of w
                    wtot = {w: int(ncols[w, :].sum()) for w in range(w0, w1)}
                    wk_ctr = {w: 0 for w in range(w0, w1)}
                    pts = {w: psw.tile([P, F], f32, tag="aggps",
                                       name=f"aggps{w}")
                           for w in range(w0, w1) if wtot[w] > 0}

                    for g in range(4):
                        rcols = int(ncols[w0:w1, g].sum())
                        if rcols == 0:
                            continue
                        o16 = int(region_off16[s, g]) - span_off16
                        rcol0 = int(region_col0[s, g])
                        # gather sub-calls of <=CALLCOLS columns
                        tiles = []      # (first_col_in_region, ncols, tile)
                        for c0 in range(0, rcols, CALLCOLS):
                            c1 = min(c0 + CALLCOLS, rcols)
                            nidx = (c1 - c0) * P
                            gt = gp.tile([P, (c1 - c0) * P], bf16, tag="g")
                            nc.gpsimd.dma_gather(
                                out_ap=gt[:].rearrange("p (s f) -> p s f", f=P),
                                in_ap=tab_in_aps[g],
                                idxs_ap=ixt[:, o16 + c0 * 8:o16 + c1 * 8],
                                num_idxs=nidx, num_idxs_reg=nidx,
                                elem_size=P, queue_num=qctr[0] % 4,
                                single_packet=False)
                            qctr[0] += 1
                            tiles.append((c0, c1 - c0, gt))
                        # one-hot matmuls for this group's columns
                        creg = 0
                        for w in range(w0, w1):
                            n = int(ncols[w, g])
                            for c in range(n):
                                colg = rcol0 + creg + c
                                ti = (creg + c) // CALLCOLS
                                toff = (creg + c) % CALLCOLS
                                gt = tiles[ti][2]
                                oh = ohp.tile([P, P], bf16, tag="oh")
                                nc.vector.tensor_scalar(
                                    out=oh[:], in0=iota_bf[:],
                                    scalar1=dpart[:, colg:colg + 1],
                                    scalar2=None,
                                    op0=mybir.AluOpType.is_equal)
                                k = wk_ctr[w]
                                nc.tensor.matmul(
                                    out=pts[w][:], lhsT=oh[:],
                                    rhs=gt[:, toff * P:toff * P + F],
                                    start=(k == 0), stop=(k == wtot[w] - 1))
                                wk_ctr[w] = k + 1
                            creg += n
                    for w in range(w0, w1):
                        if wtot[w] > 0:
                            out_cb(w, pts[w])

            # table input APs per group
            def tabP_aps(t):
                return [t[q * CHP:(q + 1) * CHP, :] for q in range(4)]

            def tabL_aps(t):
                return [t[q:q + 4 * CHP, :].rearrange(
                    "(r k) f -> r (k f)", k=4) for q in range(4)]

            # ---- stage matmuls: u' = v @ W -> streamed bf16 bounce writes ----
            def stage_matmul(vtile, F_in, W_ap, F_out, bn, ub_off):
                per = min(P // F_in, 3)
                for wb in range(0, NWIN, per):
                    nwt = min(per, NWIN - wb)
                    tp = ps.tile([P, P], f32, tag="tps")
                    nc.tensor.transpose(
                        out=tp[0:nwt * F_in, :],
                        in_=vtile[:, wb * F_in:(wb + nwt) * F_in],
                        identity=ident[:])
                    vT = wk.tile([P, P], f32, tag="vT")
                    nc.scalar.copy(out=vT[0:nwt * F_in, :], in_=tp[0:nwt * F_in, :])
                    ublk = wk.tile([P, nwt * F_out], bf16, tag="ub")
                    for a in range(nwt):
                        up = psu.tile([P, F_out], f32, tag="ups")
                        nc.tensor.matmul(out=up[:],
                                         lhsT=vT[a * F_in:(a + 1) * F_in, :],
                                         rhs=W_ap[a * F_in:(a + 1) * F_in, :],
                                         start=True, stop=True)
                        nc.scalar.copy(out=ublk[:, a * F_out:(a + 1) * F_out],
                                       in_=up[:])
                    nc.sync.dma_start(
                        out=bn[wb * P:(wb + nwt) * P, ub_off:ub_off + F_out]
                            .rearrange("(w p) f -> p w f", p=P),
                        in_=ublk[:].rearrange("p (w f) -> p w f", w=nwt))

            def allgather(bn, tab):
                nc.gpsimd.collective_compute(
                    "AllGather", bass.mybir.AluOpType.bypass,
                    replica_groups=[list(range(NC))],
                    ins=[bn[:, :].opt()],
                    outs=[tab[0:TABROWS, :].opt()])

            # ---- initial tables: uf1 = (dinv*x)@W0, ul1 = dinvy@Wl0 ----
            XB = 4
            for wb in range(0, NWIN, XB):
                nwt = min(XB, NWIN - wb)
                ublk = wk.tile([P, nwt * HID], bf16, tag="ubx")
                for a in range(nwt):
                    w = wb + a
                    xt = wk.tile([P, IN_DIM], f32, tag="xt")
                    nc.sync.dma_start(out=xt[:], in_=x_sh[w * P:(w + 1) * P, :])
                    nc.vector.tensor_scalar(out=xt[:], in0=xt[:],
                                            scalar1=dinvb[:, w:w + 1],
                                            scalar2=None,
                                            op0=mybir.AluOpType.mult)
                    tp = ps.tile([P, P], f32, tag="tps")
                    nc.tensor.transpose(out=tp[:], in_=xt[:], identity=ident[:])
                    vT = wk.tile([P, P], f32, tag="vT")
                    nc.scalar.copy(out=vT[:], in_=tp[:])
                    up = psu.tile([P, HID], f32, tag="ups")
                    nc.tensor.matmul(out=up[:], lhsT=vT[:], rhs=W0[:],
                                     start=True, stop=True)
                    nc.scalar.copy(out=ublk[:, a * HID:(a + 1) * HID], in_=up[:])
                nc.sync.dma_start(
                    out=bnP[0][wb * P:(wb + nwt) * P, 0:HID]
                        .rearrange("(w p) f -> p w f", p=P),
                    in_=ublk[:].rearrange("p (w f) -> p w f", w=nwt))
            stage_matmul(dinvy, OUT, Wl[:, 0:OUT], OUT, bnP[0], HID)
            allgather(bnP[0], tabP[0])

            # ---- round r0 (paired: feature conv1 + label conv1) ----
            def cb_r0(w, pt):
                dstF = vF[:, w * HID:(w + 1) * HID]
                nc.vector.tensor_scalar(out=dstF, in0=pt[:, 0:HID],
                                        scalar1=recipb[:, w:w + 1], scalar2=None,
                                        op0=mybir.AluOpType.mult)
                if bias0 is not None:
                    dv = wk.tile([P, HID], f32, tag="dbv")
                    nc.vector.tensor_scalar(out=dv[:], in0=bias0[:],
                                            scalar1=dinvb[:, w:w + 1],
                                            scalar2=None,
                                            op0=mybir.AluOpType.mult)
                    nc.vector.tensor_add(out=dstF, in0=dstF, in1=dv[:])
                dstL = vL[:, w * OUT:(w + 1) * OUT]
                nc.vector.tensor_scalar(out=dstL, in0=pt[:, HID:HID + OUT],
                                        scalar1=recipb[:, w:w + 1], scalar2=None,
                                        op0=mybir.AluOpType.mult)
                nc.vector.copy_predicated(
                    out=dstL, mask=maskb[:, w:w + 1].to_broadcast([P, OUT]),
                    data=dinvy[:, w * OUT:(w + 1) * OUT])
            stage_agg(layP, tabP_aps(tabP[0]), idxP_d, HID + OUT, cb_r0)

            # tables for r1: uf2 = vF@W1, ul2 = vL@Wl1
            stage_matmul(vF, HID, W1[:, :], HID, bnP[1], 0)
            stage_matmul(vL, OUT, Wl[:, OUT:2 * OUT], OUT, bnP[1], HID)
            allgather(bnP[1], tabP[1])

            # ---- round r1 (paired: feature conv2 final + label conv2) ----
            def cb_r1(w, pt):
                dstF = hfin[:, w * HID:(w + 1) * HID]
                nc.vector.tensor_scalar(out=dstF, in0=pt[:, 0:HID],
                                        scalar1=dinvb[:, w:w + 1], scalar2=None,
                                        op0=mybir.AluOpType.mult)
                if bias1 is not None:
                    nc.vector.tensor_add(out=dstF, in0=dstF, in1=bias1[:])
                dstL = vL[:, w * OUT:(w + 1) * OUT]
                nc.vector.tensor_scalar(out=dstL, in0=pt[:, HID:HID + OUT],
                                        scalar1=recipb[:, w:w + 1], scalar2=None,
                                        op0=mybir.AluOpType.mult)
                nc.vector.copy_predicated(
                    out=dstL, mask=maskb[:, w:w + 1].to_broadcast([P, OUT]),
                    data=dinvy[:, w * OUT:(w + 1) * OUT])
            stage_agg(layP, tabP_aps(tabP[1]), idxP_d, HID + OUT, cb_r1)

            # switch dpart to the label layout (paired layout now dead)
            nc.sync.dma_start(out=dpart[:, 0:layL["totcols"]],
                              in_=dpartL_d[:, :])

            # ---- label-only rounds: convs 3..10 ----
            # table for conv j (1-indexed) is ul_j = vL @ Wl[j-1]
            stage_matmul(vL, OUT, Wl[:, 2 * OUT:3 * OUT], OUT, bnL[0], 0)
            allgather(bnL[0], tabL[0])

            for j in range(3, NUM_LBL + 1):
                last = (j == NUM_LBL)
                ti = (j - 3) % 2

                def cb_lbl(w, pt, last=last):
                    if last:
                        dst = xlfin[:, w * OUT:(w + 1) * OUT]
                        nc.vector.tensor_scalar(out=dst, in0=pt[:, 0:OUT],
                                                scalar1=dinvb[:, w:w + 1],
                                                scalar2=None,
                                                op0=mybir.AluOpType.mult)
                        nc.vector.copy_predicated(
                            out=dst,
                            mask=maskb[:, w:w + 1].to_broadcast([P, OUT]),
                            data=yb[:, w * OUT:(w + 1) * OUT])
                    else:
                        dst = vL[:, w * OUT:(w + 1) * OUT]
                        nc.vector.tensor_scalar(out=dst, in0=pt[:, 0:OUT],
                                                scalar1=recipb[:, w:w + 1],
                                                scalar2=None,
                                                op0=mybir.AluOpType.mult)
                        nc.vector.copy_predicated(
                            out=dst,
                            mask=maskb[:, w:w + 1].to_broadcast([P, OUT]),
                            data=dinvy[:, w * OUT:(w + 1) * OUT])

                stage_agg(layL, tabL_aps(tabL[ti]), idxL_d, OUT, cb_lbl)
                if not last:
                    stage_matmul(vL, OUT, Wl[:, j * OUT:(j + 1) * OUT], OUT,
                                 bnL[(ti + 1) % 2], 0)
                    allgather(bnL[(ti + 1) % 2], tabL[(ti + 1) % 2])

            # ---- fuse: sigmoid([h | xl | dw] @ Wf + bf) ----
            oblk = None
            for w in range(NWIN):
                if w % XB == 0:
                    nblk = min(XB, NWIN - w)
                    oblk = wk.tile([P, nblk * OUT], f32, tag="ofin")
                dwt = wk.tile([P, DW], f32, tag="dwt")
                nc.sync.dma_start(out=dwt[:], in_=dw_sh[w * P:(w + 1) * P, :])
                fTa = wk.tile([P, P], f32, tag="fTa")
                fTb = wk.tile([DW - 32, P], f32, tag="fTb")
                tp = ps.tile([P, P], f32, tag="tps")
                nc.tensor.transpose(out=tp[0:HID, :],
                                    in_=hfin[:, w * HID:(w + 1) * HID],
                                    identity=ident[:])
                nc.scalar.copy(out=fTa[0:HID, :], in_=tp[0:HID, :])
                tp2 = ps.tile([P, P], f32, tag="tps")
                nc.tensor.transpose(out=tp2[0:OUT, :],
                                    in_=xlfin[:, w * OUT:(w + 1) * OUT],
                                    identity=ident[:])
                nc.scalar.copy(out=fTa[HID:HID + OUT, :], in_=tp2[0:OUT, :])
                tp3 = ps.tile([P, P], f32, tag="tps")
                nc.tensor.transpose(out=tp3[0:DW, :], in_=dwt[:],
                                    identity=ident[:])
                nc.scalar.copy(out=fTa[HID + OUT:P, :],
                               in_=tp3[0:P - HID - OUT, :])
                nc.scalar.copy(out=fTb[:, :], in_=tp3[P - HID - OUT:DW, :])
                op = psu.tile([P, OUT], f32, tag="ups", name="ops")
                nc.tensor.matmul(out=op[:], lhsT=fTa[:], rhs=Wfa[:],
                                 start=True, stop=False)
                nc.tensor.matmul(out=op[:], lhsT=fTb[:], rhs=Wfb[:],
                                 start=False, stop=True)
                if biasf is not None:
                    nc.vector.tensor_add(out=op[:], in0=op[:], in1=biasf[:])
                nc.scalar.activation(out=oblk[:, (w % XB) * OUT:
                                              (w % XB + 1) * OUT],
                                     in_=op[:],
                                     func=bass.mybir.ActivationFunctionType.Sigmoid)
                if w % XB == XB - 1 or w == NWIN - 1:
                    wb = (w // XB) * XB
                    nblk = w - wb + 1
                    nc.sync.dma_start(
                        out=out_sh[wb * P:(w + 1) * P, :]
                            .rearrange("(w p) f -> p w f", p=P),
                        in_=oblk[:].rearrange("p (w f) -> p w f", w=nblk))

    nc.compile()
    return nc


_CACHE = {}


def kernel(x, y, edge_index, deep_walk_emb, label_input_mask,
           W_gcn0, b_gcn0, W_gcn1, b_gcn1, W_label, b_label, W_fuse, b_fuse):
    import concourse.bass_utils as bass_utils
    import ml_dtypes

    n_nodes = x.shape[0]
    ei = np.asarray(edge_index, dtype=np.int64)
    meta = _preprocess(ei, n_nodes)
    core_of, local_of = meta["core_of"], meta["local_of"]
    layP, layL = meta["layP"], meta["layL"]

    nonzero_b = (bool(np.any(np.asarray(b_gcn0))),
                 bool(np.any(np.asarray(b_gcn1))),
                 bool(np.any(np.asarray(b_label))),
                 bool(np.any(np.asarray(b_fuse))))
    if nonzero_b[2]:
        raise NotImplementedError("nonzero label bias not wired")

    key = ("k2", n_nodes, ei.shape[1], nonzero_b,
           layP["totcols"], layL["totcols"])
    if key not in _CACHE:
        _CACHE[key] = _build(meta, nonzero_b)
    nc = _CACHE[key]

    x_s = _shard_nodes(x, core_of, local_of, IN_DIM)
    y_s = _shard_nodes(y, core_of, local_of, OUT)
    dw_s = _shard_nodes(deep_walk_emb, core_of, local_of, DW)
    mk_s = _shard_nodes(np.asarray(label_input_mask, np.int8)[:, None],
                        core_of, local_of, 1, dtype=np.int8)
    dg_s = np.zeros((NC, NPAD, 1), np.int32)
    dg_s[core_of, local_of, 0] = meta["deg"].astype(np.int32)

    bmax = max(HID, OUT) * NUM_LBL
    b_all = np.zeros((4, bmax), np.float32)
    b_all[0, :HID] = np.asarray(b_gcn0, np.float32)
    b_all[1, :HID] = np.asarray(b_gcn1, np.float32)
    b_all[2, :OUT * NUM_LBL] = np.asarray(b_label, np.float32).reshape(-1)
    b_all[3, :OUT] = np.asarray(b_fuse, np.float32)

    Wl_flat = np.asarray(W_label, np.float32).reshape(NUM_LBL * OUT, OUT)
    idxP128 = np.tile(layP["idx16"], (1, 8, 1))
    idxL128 = np.tile(layL["idx16"], (1, 8, 1))
    dpP = layP["dpart"]
    dpL = layL["dpart"]

    in_maps = []
    for c in range(NC):
        in_maps.append({
            "x_sh": x_s[c], "y_sh": y_s[c], "dw_sh": dw_s[c],
            "mask_sh": mk_s[c], "deg_sh": dg_s[c],
            "idxP_d": idxP128[c], "idxL_d": idxL128[c],
            "dpartP_d": dpP[c], "dpartL_d": dpL[c],
            "W0": np.asarray(W_gcn0, np.float32),
            "W1": np.asarray(W_gcn1, np.float32),
            "Wl": Wl_flat,
            "Wf": np.asarray(W_fuse, np.float32),
            "b_all": b_all,
        })
    res = bass_utils.run_bass_kernel_spmd(nc, in_maps, core_ids=list(range(NC)))
    out = np.empty((n_nodes, OUT), np.float32)
    for c in range(NC):
        sel = core_of == np.int64(c)
        out[sel] = res.results[c]["out_sh"][local_of[sel]]
    return out


# revision 26
# speedup vs baseline: 3.8103x; 2.0623x over previous
"""FPLPGCN (2x GCNConv feature prop + 10x label prop + fuse) on 8 trn2 cores.

Strategy (graph/data parallel):
- Nodes sorted by in-degree, striped round-robin across 8 cores.  Each core
  owns NPAD=12544 rows (44 zero "fake" rows pad 100000 -> 100352).
- GCN refactor: out[n] = dinv[n]*(sum_{e->n} u'[src_e] + u'[n]) + b with
  u' = dinv*(z @ W).  Self term is appended as a regular (n,n) edge.
- Tables are bf16 and AllGather'ed each round.  Rounds 1-2 use a paired
  table [u_feat 64 | u_label 32 | pad 32] so one gather serves both chains
  (10 gather rounds total instead of 12).
- Aggregation: dense-packed dma_gather slots (256B elems), then per
  128-slot column a one-hot (dst-partition) matrix is built on DVE
  (is_equal vs iota) and the PE matmul-accumulates messages into per-window
  PSUM.  No per-node slot padding -> ~2x fewer descriptors than the
  node-aligned layout.
"""

import sys

sys.path.insert(0, "/opt/trn_rl_repo")

import numpy as np

NC = 8
P = 128
NPAD = 12544           # local rows per core (98 windows of 128)
NWIN = NPAD // P       # 98
TABROWS = NC * NPAD    # 100352
CHP = TABROWS // 4     # 25088: paired-layout chunk (row stride 256B)
IN_DIM, HID, OUT, DW = 128, 64, 32, 64
NUM_LBL = 10
SPAN_P = 5             # windows per gather-call group (paired layout)
SPAN_L = 7             # windows per gather-call group (label layout)
MAXCOLS = 28           # <=3584 idxs per dma_gather call


def _cdiv(a, b):
    return -(-a // b)


# ----------------------------------------------------------------------------
# host-side index preprocessing (pure index manipulation; no FP math on data)
# ----------------------------------------------------------------------------

def _build_layout(ecore, ewin, epart, grp, idxv, span):
    """Dense slot layout for one gather addressing scheme.

    Returns dict with per-core idx16 + dpart buffers and the static column
    tables used to emit the device program.
    """
    EA = ecore.shape[0]
    # counts per (core, win, grp)
    key = (ecore * NWIN + ewin) * 4 + grp
    cnt = np.bincount(key, minlength=NC * NWIN * 4).reshape(NC, NWIN, 4)
    ncols = _cdiv(cnt.max(axis=0), 128)                     # [NWIN, 4]

    spans = [(w0, min(w0 + span, NWIN)) for w0 in range(0, NWIN, span)]
    # global column allocation: (span, grp, win) order
    col_start = np.zeros((NWIN, 4), np.int64)
    region_col0 = np.zeros((len(spans), 4), np.int64)       # first col of region
    region_off16 = np.zeros((len(spans), 4), np.int64)      # idx16 col offset
    col = 0
    off16 = 0
    for s, (w0, w1) in enumerate(spans):
        for g in range(4):
            region_col0[s, g] = col
            region_off16[s, g] = off16
            for w in range(w0, w1):
                col_start[w, g] = col
                col += int(ncols[w, g])
            off16 += int((col - region_col0[s, g]) * 8)
    totcols = col
    tot16 = off16

    # per-edge slot position within its (core, win, grp) group
    o = np.argsort(key, kind="stable")
    ks = key[o]
    first = np.searchsorted(ks, ks, side="left")
    pos = np.empty(EA, np.int64)
    pos[o] = np.arange(EA) - first

    span_of_w = np.arange(NWIN) // span
    ecolg = col_start[ewin, grp] + pos // 128
    part = pos % 128
    es = span_of_w[ewin]
    i_in_region = (ecolg - region_col0[es, grp]) * 128 + part

    idx16 = np.zeros((NC, 16, tot16), np.int16)
    idx16[ecore, i_in_region % 16,
          region_off16[es, grp] + i_in_region // 16] = idxv.astype(np.int16)

    dpart = np.full((NC, 128, totcols), 128.0, np.float32)
    dpart[ecore, part, ecolg] = epart.astype(np.float32)

    # static tables for program emission
    return dict(idx16=idx16, dpart=dpart, ncols=ncols, spans=spans, span=span,
                region_col0=region_col0, region_off16=region_off16,
                totcols=totcols, tot16=tot16)


def _preprocess(edge_index, mask, n_nodes):
    src = np.ascontiguousarray(edge_index[0]).astype(np.int64)
    dst = np.ascontiguousarray(edge_index[1]).astype(np.int64)
    deg = np.bincount(dst, minlength=n_nodes).astype(np.int64)
    mask = np.ascontiguousarray(mask).astype(bool)

    # masked nodes (label overwritten by y) first, then unmasked; degree-sorted
    # within each group.  Label aggregation only touches unmasked dst nodes,
    # which cluster into the high windows.
    order = np.lexsort((deg, (~mask).astype(np.int64)))
    rank = np.empty(n_nodes, np.int64)
    rank[order] = np.arange(TABROWS - n_nodes, TABROWS)  # fakes get ranks 0..351
    core_of = rank % NC
    local_of = rank // NC
    trow = core_of * NPAD + local_of                # table row per real node

    # paired layout: all edges + self edge (n, n) for every real node
    s_all = np.concatenate([src, np.arange(n_nodes, dtype=np.int64)])
    d_all = np.concatenate([dst, np.arange(n_nodes, dtype=np.int64)])

    def mk(lay_s, lay_d, grp_fn, idx_fn, span):
        etr = trow[lay_s]
        eloc = local_of[lay_d]
        return _build_layout(core_of[lay_d], eloc // P, eloc % P,
                             grp_fn(etr), idx_fn(etr), span)

    layP = mk(s_all, d_all, lambda t: t // CHP, lambda t: t % CHP, SPAN_P)

    # label layout: only edges whose dst is unmasked (+ their self edges)
    selE = ~mask[dst]
    un = np.flatnonzero(~mask)
    sL = np.concatenate([src[selE], un])
    dL = np.concatenate([dst[selE], un])
    layL = mk(sL, dL, lambda t: t % 4, lambda t: t // 4, SPAN_L)

    meta = dict(order=order, core_of=core_of, local_of=local_of, trow=trow,
                deg=deg, layP=layP, layL=layL)
    return meta


def _shard_nodes(arr, core_of, local_of, width, dtype=np.float32):
    n = arr.shape[0]
    out = np.zeros((NC, NPAD, width), dtype)
    a2 = np.asarray(arr, dtype).reshape(n, width)
    out[core_of, local_of] = a2
    return out


# ----------------------------------------------------------------------------
# device program
# ----------------------------------------------------------------------------

def _build(meta, nonzero_b):
    import concourse.bacc as bacc
    import concourse.bass as bass
    import concourse.mybir as mybir
    import concourse.tile as tile

    f32 = mybir.dt.float32
    bf16 = mybir.dt.bfloat16
    layP, layL = meta["layP"], meta["layL"]

    TABP_ALLOC = TABROWS           # paired rows: elem = full row, no spill
    TABL_ALLOC = TABROWS + 8       # label: elem spans 4 rows -> slack

    nc = bacc.Bacc("TRN2", target_bir_lowering=False, debug=False,
                   num_devices=NC, num_swdge_queues=4)

    x_sh = nc.dram_tensor("x_sh", [NPAD, IN_DIM], f32, kind="ExternalInput")
    y_sh = nc.dram_tensor("y_sh", [NPAD, OUT], f32, kind="ExternalInput")
    dw_sh = nc.dram_tensor("dw_sh", [NPAD, DW], f32, kind="ExternalInput")
    mask_sh = nc.dram_tensor("mask_sh", [NPAD, 1], mybir.dt.int8,
                             kind="ExternalInput")
    deg_sh = nc.dram_tensor("deg_sh", [NPAD, 1], mybir.dt.int32,
                            kind="ExternalInput")
    idxP_d = nc.dram_tensor("idxP_d", [P, layP["tot16"]], mybir.dt.int16,
                            kind="ExternalInput")
    idxL_d = nc.dram_tensor("idxL_d", [P, layL["tot16"]], mybir.dt.int16,
                            kind="ExternalInput")
    ohP_d = nc.dram_tensor("ohP_d", [P, layP["totcols"] * P], bf16,
                           kind="ExternalInput")
    ohL_d = nc.dram_tensor("ohL_d", [P, layL["totcols"] * P], bf16,
                           kind="ExternalInput")
    W0_d = nc.dram_tensor("W0", [IN_DIM, HID], f32, kind="ExternalInput")
    W1_d = nc.dram_tensor("W1", [HID, HID], f32, kind="ExternalInput")
    Wl_d = nc.dram_tensor("Wl", [NUM_LBL * OUT, OUT], f32, kind="ExternalInput")
    Wf_d = nc.dram_tensor("Wf", [HID + OUT + DW, OUT], f32, kind="ExternalInput")
    b_d = nc.dram_tensor("b_all", [4, max(HID, OUT) * NUM_LBL], f32,
                         kind="ExternalInput")
    out_sh = nc.dram_tensor("out_sh", [NPAD, OUT], f32, kind="ExternalOutput")

    # internal DRAM
    tabP = [nc.dram_tensor(f"tabP{i}", [TABP_ALLOC, 2 * HID], bf16,
                           addr_space="Shared") for i in range(2)]
    tabL = [nc.dram_tensor(f"tabL{i}", [TABL_ALLOC, OUT], bf16,
                           addr_space="Shared") for i in range(2)]
    bnP = [nc.dram_tensor(f"bnP{i}", [NPAD, 2 * HID], bf16) for i in range(2)]
    bnL = [nc.dram_tensor(f"bnL{i}", [NPAD, OUT], bf16) for i in range(2)]

    with tile.TileContext(nc) as tc:
        with tc.tile_pool(name="persist", bufs=1) as pp, \
             tc.tile_pool(name="g", bufs=6) as gp, \
             tc.tile_pool(name="ix", bufs=3) as ixp, \
             tc.tile_pool(name="oh", bufs=3) as ohp, \
             tc.tile_pool(name="wk", bufs=3) as wk, \
             tc.tile_pool(name="ps", bufs=2, space="PSUM") as ps, \
             tc.tile_pool(name="psu", bufs=2, space="PSUM") as psu, \
             tc.tile_pool(name="psw", bufs=4, space="PSUM") as psw:

            # ---- constants / persistent state ----
            W0 = pp.tile([IN_DIM, HID], f32)
            nc.sync.dma_start(out=W0[:], in_=W0_d[:, :])
            W1 = pp.tile([P, HID], f32)
            for a in range(P // HID):
                nc.sync.dma_start(out=W1[a * HID:(a + 1) * HID, :], in_=W1_d[:, :])
            Wl = pp.tile([P, NUM_LBL * OUT], f32)
            for j in range(NUM_LBL):
                for a in range(3):  # lhsT base partitions 0/32/64
                    nc.sync.dma_start(
                        out=Wl[a * OUT:(a + 1) * OUT, j * OUT:(j + 1) * OUT],
                        in_=Wl_d[j * OUT:(j + 1) * OUT, :])
            Wfa = pp.tile([128, OUT], f32)
            nc.sync.dma_start(out=Wfa[:], in_=Wf_d[0:128, :])
            Wfb = pp.tile([HID + OUT + DW - 128, OUT], f32)
            nc.sync.dma_start(out=Wfb[:], in_=Wf_d[128:, :])
            from concourse.masks import make_identity
            ident = pp.tile([P, P], f32)
            make_identity(nc, ident[:])

            onecol = pp.tile([1, P], f32)
            nc.vector.memset(onecol[:], 1.0)

            yb = pp.tile([P, NWIN * OUT], f32)
            nc.sync.dma_start(
                out=yb[:].rearrange("p (w f) -> p w f", w=NWIN),
                in_=y_sh[:, :].rearrange("(w p) f -> p w f", p=P))
            maskb = pp.tile([P, NWIN], mybir.dt.int8)
            nc.sync.dma_start(
                out=maskb[:], in_=mask_sh[:, 0].rearrange("(w p) -> p w", p=P))
            degb = pp.tile([P, NWIN], mybir.dt.int32)
            nc.sync.dma_start(
                out=degb[:], in_=deg_sh[:, 0].rearrange("(w p) -> p w", p=P))

            degf = pp.tile([P, NWIN], f32)
            nc.vector.tensor_copy(out=degf[:], in_=degb[:])
            recipb = pp.tile([P, NWIN], f32)
            nc.vector.tensor_scalar(out=degf[:], in0=degf[:], scalar1=1.0,
                                    scalar2=None, op0=mybir.AluOpType.add)
            nc.vector.reciprocal(out=recipb[:], in_=degf[:])      # 1/(deg+1)
            dinvb = pp.tile([P, NWIN], f32)
            nc.scalar.sqrt(out=dinvb[:], in_=recipb[:])           # 1/sqrt(deg+1)
            # zero fake lanes (window 0, partitions 0..43)
            nfake = TABROWS - 100000
            nc.vector.memset(recipb[0:nfake // NC, 0:1], 0.0)
            nc.vector.memset(dinvb[0:nfake // NC, 0:1], 0.0)
            dinvy = pp.tile([P, NWIN * OUT], f32)
            for w in range(NWIN):
                nc.vector.tensor_scalar(
                    out=dinvy[:, w * OUT:(w + 1) * OUT],
                    in0=yb[:, w * OUT:(w + 1) * OUT],
                    scalar1=dinvb[:, w:w + 1], scalar2=None,
                    op0=mybir.AluOpType.mult)

            def bias_tile(row, width):
                bt = pp.tile([P, width], f32, tag=f"bias{row}", name=f"bias{row}")
                brow = pp.tile([1, width], f32, tag=f"brow{row}", name=f"brow{row}")
                nc.sync.dma_start(out=brow[:], in_=b_d[row:row + 1, 0:width])
                pt = ps.tile([P, P], f32, tag="tps", name="biasps")
                nc.tensor.matmul(out=pt[:, 0:width], lhsT=onecol[:],
                                 rhs=brow[:], start=True, stop=True)
                nc.vector.tensor_copy(out=bt[:], in_=pt[:, 0:width])
                return bt

            bias0 = bias_tile(0, HID) if nonzero_b[0] else None
            bias1 = bias_tile(1, HID) if nonzero_b[1] else None
            biasf = bias_tile(3, OUT) if nonzero_b[3] else None

            vF = pp.tile([P, NWIN * HID], f32)      # next-round feature input
            vL = pp.tile([P, NWIN * OUT], f32)      # next-round label input
            hfin = pp.tile([P, NWIN * HID], f32)
            xlfin = pp.tile([P, NWIN * OUT], f32)

            # ---- aggregation engine ----
            def stage_agg(lay, tab_in_aps, idx_d, oh_d, F, out_cb):
                """Gather + one-hot matmul segment-sum.

                out_cb(w, psum_slice_ap) for each window with work; the
                one-hot matrices come precomputed from DRAM.
                """
                ncols = lay["ncols"]
                spans = lay["spans"]
                region_col0 = lay["region_col0"]
                region_off16 = lay["region_off16"]
                qctr = [0]
                for s, (w0, w1) in enumerate(spans):
                    span_off16 = int(region_off16[s, 0])
                    span_cols = int(ncols[w0:w1, :].sum())
                    if span_cols == 0:
                        continue
                    span_n16 = span_cols * 8
                    ixt = ixp.tile([P, span_n16], mybir.dt.int16, tag="ix")
                    nc.sync.dma_start(
                        out=ixt[:], in_=idx_d[:, span_off16:span_off16 + span_n16])

                    # Chain bookkeeping.  start=True clears has_written for the
                    # WHOLE psum bank, so only the very first matmul into this
                    # span tile may set it; later windows start fresh via their
                    # regions' unset has_written bits.
                    wtot = {w: int(ncols[w, :].sum()) for w in range(w0, w1)}
                    wk_ctr = {w: 0 for w in range(w0, w1)}
                    nw = w1 - w0
                    pt = psw.tile([P, nw * F], f32, tag="aggps", name="aggps")
                    kspan = [0]
                    kspan_tot = int(sum(wtot.values()))

                    for g in range(4):
                        rcols = int(ncols[w0:w1, g].sum())
                        if rcols == 0:
                            continue
                        o16 = int(region_off16[s, g]) - span_off16
                        rcol0 = int(region_col0[s, g])
                        ohrt = ohp.tile([P, rcols * P], bf16, tag="oh")
                        nc.sync.dma_start(
                            out=ohrt[:],
                            in_=oh_d[:, rcol0 * P:(rcol0 + rcols) * P])
                        # gather sub-calls of <=MAXCOLS columns
                        tiles = []
                        for c0 in range(0, rcols, MAXCOLS):
                            c1 = min(c0 + MAXCOLS, rcols)
                            nidx = (c1 - c0) * P
                            gt = gp.tile([P, (c1 - c0) * P], bf16, tag="g")
                            nc.gpsimd.dma_gather(
                                out_ap=gt[:].rearrange("p (s f) -> p s f", f=P),
                                in_ap=tab_in_aps[g],
                                idxs_ap=ixt[:, o16 + c0 * 8:o16 + c1 * 8],
                                num_idxs=nidx, num_idxs_reg=nidx,
                                elem_size=P, queue_num=qctr[0] % 4,
                                single_packet=False)
                            qctr[0] += 1
                            tiles.append(gt)
                        # one-hot matmuls for this group's columns
                        creg = 0
                        for w in range(w0, w1):
                            n = int(ncols[w, g])
                            pslice = pt[:, (w - w0) * F:(w - w0 + 1) * F]
                            for c in range(n):
                                cr = creg + c
                                gt = tiles[cr // MAXCOLS]
                                toff = cr % MAXCOLS
                                ks = kspan[0]
                                nc.tensor.matmul(
                                    out=pslice, lhsT=ohrt[:, cr * P:(cr + 1) * P],
                                    rhs=gt[:, toff * P:toff * P + F],
                                    start=(ks == 0),
                                    stop=(ks == kspan_tot - 1))
                                kspan[0] = ks + 1
                                wk_ctr[w] += 1
                            creg += n
                    for w in range(w0, w1):
                        if wtot[w] > 0:
                            out_cb(w, pt, (w - w0) * F)

            # table input APs per group
            def tabP_aps(t):
                return [t[q * CHP:(q + 1) * CHP, :] for q in range(4)]

            def tabL_aps(t):
                return [t[q:q + 4 * CHP, :].rearrange(
                    "(r k) f -> r (k f)", k=4) for q in range(4)]

            # ---- stage matmuls: u' = v @ W -> streamed bf16 bounce writes ----
            def stage_matmul(vtile, F_in, W_ap, F_out, bn, ub_off):
                per = min(P // F_in, 3)
                for wb in range(0, NWIN, per):
                    nwt = min(per, NWIN - wb)
                    tp = ps.tile([P, P], f32, tag="tps")
                    nc.tensor.transpose(
                        out=tp[0:nwt * F_in, :],
                        in_=vtile[:, wb * F_in:(wb + nwt) * F_in],
                        identity=ident[:])
                    vT = wk.tile([P, P], f32, tag="vT")
                    nc.scalar.copy(out=vT[0:nwt * F_in, :], in_=tp[0:nwt * F_in, :])
                    ublk = wk.tile([P, nwt * F_out], bf16, tag="ub")
                    for a in range(nwt):
                        up = psu.tile([P, F_out], f32, tag="ups")
                        nc.tensor.matmul(out=up[:],
                                         lhsT=vT[a * F_in:(a + 1) * F_in, :],
                                         rhs=W_ap[a * F_in:(a + 1) * F_in, :],
                                         start=True, stop=True)
                        nc.scalar.copy(out=ublk[:, a * F_out:(a + 1) * F_out],
                                       in_=up[:])
                    nc.sync.dma_start(
                        out=bn[wb * P:(wb + nwt) * P, ub_off:ub_off + F_out]
                            .rearrange("(w p) f -> p w f", p=P),
                        in_=ublk[:].rearrange("p (w f) -> p w f", w=nwt))

            def allgather(bn, tab):
                nc.gpsimd.collective_compute(
                    "AllGather", bass.mybir.AluOpType.bypass,
                    replica_groups=[list(range(NC))],
                    ins=[bn[:, :].opt()],
                    outs=[tab[0:TABROWS, :].opt()])

            # ---- initial tables: uf1 = (dinv*x)@W0, ul1 = dinvy@Wl0 ----
            XB = 4
            for wb in range(0, NWIN, XB):
                nwt = min(XB, NWIN - wb)
                ublk = wk.tile([P, nwt * HID], bf16, tag="ubx")
                for a in range(nwt):
                    w = wb + a
                    xt = wk.tile([P, IN_DIM], f32, tag="xt")
                    nc.sync.dma_start(out=xt[:], in_=x_sh[w * P:(w + 1) * P, :])
                    nc.vector.tensor_scalar(out=xt[:], in0=xt[:],
                                            scalar1=dinvb[:, w:w + 1],
                                            scalar2=None,
                                            op0=mybir.AluOpType.mult)
                    tp = ps.tile([P, P], f32, tag="tps")
                    nc.tensor.transpose(out=tp[:], in_=xt[:], identity=ident[:])
                    vT = wk.tile([P, P], f32, tag="vT")
                    nc.scalar.copy(out=vT[:], in_=tp[:])
                    up = psu.tile([P, HID], f32, tag="ups")
                    nc.tensor.matmul(out=up[:], lhsT=vT[:], rhs=W0[:],
                                     start=True, stop=True)
                    nc.scalar.copy(out=ublk[:, a * HID:(a + 1) * HID], in_=up[:])
                nc.sync.dma_start(
                    out=bnP[0][wb * P:(wb + nwt) * P, 0:HID]
                        .rearrange("(w p) f -> p w f", p=P),
                    in_=ublk[:].rearrange("p (w f) -> p w f", w=nwt))
            stage_matmul(dinvy, OUT, Wl[:, 0:OUT], OUT, bnP[0], HID)
            allgather(bnP[0], tabP[0])

            # ---- round r0 (paired: feature conv1 + label conv1) ----
            def cb_r0(w, pt, b):
                dstF = vF[:, w * HID:(w + 1) * HID]
                nc.vector.tensor_scalar(out=dstF, in0=pt[:, b:b + HID],
                                        scalar1=recipb[:, w:w + 1], scalar2=None,
                                        op0=mybir.AluOpType.mult)
                if bias0 is not None:
                    dv = wk.tile([P, HID], f32, tag="dbv")
                    nc.vector.tensor_scalar(out=dv[:], in0=bias0[:],
                                            scalar1=dinvb[:, w:w + 1],
                                            scalar2=None,
                                            op0=mybir.AluOpType.mult)
                    nc.vector.tensor_add(out=dstF, in0=dstF, in1=dv[:])
                dstL = vL[:, w * OUT:(w + 1) * OUT]
                nc.vector.tensor_scalar(out=dstL, in0=pt[:, b + HID:b + HID + OUT],
                                        scalar1=recipb[:, w:w + 1], scalar2=None,
                                        op0=mybir.AluOpType.mult)
                nc.vector.copy_predicated(
                    out=dstL, mask=maskb[:, w:w + 1].to_broadcast([P, OUT]),
                    data=dinvy[:, w * OUT:(w + 1) * OUT])
            stage_agg(layP, tabP_aps(tabP[0]), idxP_d, ohP_d, HID + OUT, cb_r0)

            # tables for r1: uf2 = vF@W1, ul2 = vL@Wl1
            stage_matmul(vF, HID, W1[:, :], HID, bnP[1], 0)
            stage_matmul(vL, OUT, Wl[:, OUT:2 * OUT], OUT, bnP[1], HID)
            allgather(bnP[1], tabP[1])

            # ---- round r1 (paired: feature conv2 final + label conv2) ----
            def cb_r1(w, pt, b):
                dstF = hfin[:, w * HID:(w + 1) * HID]
                nc.vector.tensor_scalar(out=dstF, in0=pt[:, b:b + HID],
                                        scalar1=dinvb[:, w:w + 1], scalar2=None,
                                        op0=mybir.AluOpType.mult)
                if bias1 is not None:
                    nc.vector.tensor_add(out=dstF, in0=dstF, in1=bias1[:])
                dstL = vL[:, w * OUT:(w + 1) * OUT]
                nc.vector.tensor_scalar(out=dstL, in0=pt[:, b + HID:b + HID + OUT],
                                        scalar1=recipb[:, w:w + 1], scalar2=None,
                                        op0=mybir.AluOpType.mult)
                nc.vector.copy_predicated(
                    out=dstL, mask=maskb[:, w:w + 1].to_broadcast([P, OUT]),
                    data=dinvy[:, w * OUT:(w + 1) * OUT])
            stage_agg(layP, tabP_aps(tabP[1]), idxP_d, ohP_d, HID + OUT, cb_r1)

            # ---- label-only rounds: convs 3..10 ----
            # table for conv j (1-indexed) is ul_j = vL @ Wl[j-1]
            stage_matmul(vL, OUT, Wl[:, 2 * OUT:3 * OUT], OUT, bnL[0], 0)
            allgather(bnL[0], tabL[0])

            for j in range(3, NUM_LBL + 1):
                last = (j == NUM_LBL)
                ti = (j - 3) % 2

                def cb_lbl(w, pt, b, last=last):
                    if last:
                        dst = xlfin[:, w * OUT:(w + 1) * OUT]
                        nc.vector.tensor_scalar(out=dst, in0=pt[:, b:b + OUT],
                                                scalar1=dinvb[:, w:w + 1],
                                                scalar2=None,
                                                op0=mybir.AluOpType.mult)
                        nc.vector.copy_predicated(
                            out=dst,
                            mask=maskb[:, w:w + 1].to_broadcast([P, OUT]),
                            data=yb[:, w * OUT:(w + 1) * OUT])
                    else:
                        dst = vL[:, w * OUT:(w + 1) * OUT]
                        nc.vector.tensor_scalar(out=dst, in0=pt[:, b:b + OUT],
                                                scalar1=recipb[:, w:w + 1],
                                                scalar2=None,
                                                op0=mybir.AluOpType.mult)
                        nc.vector.copy_predicated(
                            out=dst,
                            mask=maskb[:, w:w + 1].to_broadcast([P, OUT]),
                            data=dinvy[:, w * OUT:(w + 1) * OUT])

                stage_agg(layL, tabL_aps(tabL[ti]), idxL_d, ohL_d, OUT, cb_lbl)
                if not last:
                    stage_matmul(vL, OUT, Wl[:, j * OUT:(j + 1) * OUT], OUT,
                                 bnL[(ti + 1) % 2], 0)
                    allgather(bnL[(ti + 1) % 2], tabL[(ti + 1) % 2])

            # fully-masked windows never get a label callback: xl = y there
            for w in range(NWIN):
                if int(layL["ncols"][w, :].sum()) == 0:
                    nc.vector.tensor_copy(
                        out=xlfin[:, w * OUT:(w + 1) * OUT],
                        in_=yb[:, w * OUT:(w + 1) * OUT])

            # ---- fuse: sigmoid([h | xl | dw] @ Wf + bf) ----
            oblk = None
            for w in range(NWIN):
                if w % XB == 0:
                    nblk = min(XB, NWIN - w)
                    oblk = wk.tile([P, nblk * OUT], f32, tag="ofin")
                dwt = wk.tile([P, DW], f32, tag="dwt")
                nc.sync.dma_start(out=dwt[:], in_=dw_sh[w * P:(w + 1) * P, :])
                fTa = wk.tile([P, P], f32, tag="fTa")
                fTb = wk.tile([DW - 32, P], f32, tag="fTb")
                tp = ps.tile([P, P], f32, tag="tps")
                nc.tensor.transpose(out=tp[0:HID, :],
                                    in_=hfin[:, w * HID:(w + 1) * HID],
                                    identity=ident[:])
                nc.scalar.copy(out=fTa[0:HID, :], in_=tp[0:HID, :])
                tp2 = ps.tile([P, P], f32, tag="tps")
                nc.tensor.transpose(out=tp2[0:OUT, :],
                                    in_=xlfin[:, w * OUT:(w + 1) * OUT],
                                    identity=ident[:])
                nc.scalar.copy(out=fTa[HID:HID + OUT, :], in_=tp2[0:OUT, :])
                tp3 = ps.tile([P, P], f32, tag="tps")
                nc.tensor.transpose(out=tp3[0:DW, :], in_=dwt[:],
                                    identity=ident[:])
                nc.scalar.copy(out=fTa[HID + OUT:P, :],
                               in_=tp3[0:P - HID - OUT, :])
                nc.scalar.copy(out=fTb[:, :], in_=tp3[P - HID - OUT:DW, :])
                op = psu.tile([P, OUT], f32, tag="ups", name="ops")
                nc.tensor.matmul(out=op[:], lhsT=fTa[:], rhs=Wfa[:],
                                 start=True, stop=False)
                nc.tensor.matmul(out=op[:], lhsT=fTb[:], rhs=Wfb[:],
                                 start=False, stop=True)
                if biasf is not None:
                    nc.vector.tensor_add(out=op[:], in0=op[:], in1=biasf[:])
                nc.scalar.activation(out=oblk[:, (w % XB) * OUT:
                                              (w % XB + 1) * OUT],
                                     in_=op[:],
                                     func=bass.mybir.ActivationFunctionType.Sigmoid)
                if w % XB == XB - 1 or w == NWIN - 1:
                    wb = (w // XB) * XB
                    nblk = w - wb + 1
                    nc.sync.dma_start(
                        out=out_sh[wb * P:(w + 1) * P, :]
                            .rearrange("(w p) f -> p w f", p=P),
                        in_=oblk[:].rearrange("p (w f) -> p w f", w=nblk))

    nc.compile()
    return nc


_CACHE = {}


def kernel(x, y, edge_index, deep_walk_emb, label_input_mask,
           W_gcn0, b_gcn0, W_gcn1, b_gcn1, W_label, b_label, W_fuse, b_fuse):
    import concourse.bass_utils as bass_utils
    import ml_dtypes

    n_nodes = x.shape[0]
    ei = np.asarray(edge_index, dtype=np.int64)
    meta = _preprocess(ei, np.asarray(label_input_mask), n_nodes)
    core_of, local_of = meta["core_of"], meta["local_of"]
    layP, layL = meta["layP"], meta["layL"]

    nonzero_b = (bool(np.any(np.asarray(b_gcn0))),
                 bool(np.any(np.asarray(b_gcn1))),
                 bool(np.any(np.asarray(b_label))),
                 bool(np.any(np.asarray(b_fuse))))
    if nonzero_b[2]:
        raise NotImplementedError("nonzero label bias not wired")

    key = ("k3", n_nodes, ei.shape[1], nonzero_b,
           layP["totcols"], layL["totcols"],
           layP["ncols"].tobytes(), layL["ncols"].tobytes())
    if key not in _CACHE:
        _CACHE[key] = _build(meta, nonzero_b)
    nc = _CACHE[key]

    x_s = _shard_nodes(x, core_of, local_of, IN_DIM)
    y_s = _shard_nodes(y, core_of, local_of, OUT)
    dw_s = _shard_nodes(deep_walk_emb, core_of, local_of, DW)
    mk_s = _shard_nodes(np.asarray(label_input_mask, np.int8)[:, None],
                        core_of, local_of, 1, dtype=np.int8)
    dg_s = np.zeros((NC, NPAD, 1), np.int32)
    dg_s[core_of, local_of, 0] = meta["deg"].astype(np.int32)

    bmax = max(HID, OUT) * NUM_LBL
    b_all = np.zeros((4, bmax), np.float32)
    b_all[0, :HID] = np.asarray(b_gcn0, np.float32)
    b_all[1, :HID] = np.asarray(b_gcn1, np.float32)
    b_all[2, :OUT * NUM_LBL] = np.asarray(b_label, np.float32).reshape(-1)
    b_all[3, :OUT] = np.asarray(b_fuse, np.float32)

    Wl_flat = np.asarray(W_label, np.float32).reshape(NUM_LBL * OUT, OUT)
    idxP128 = np.tile(layP["idx16"], (1, 8, 1))
    idxL128 = np.tile(layL["idx16"], (1, 8, 1))

    def onehots(dpart):
        ncc, pp_, tcc = dpart.shape
        out = np.empty((ncc, pp_, tcc * 128), ml_dtypes.bfloat16)
        ar = np.arange(128, dtype=np.float32)
        for c in range(ncc):
            out[c] = (dpart[c][:, :, None] == ar).reshape(
                pp_, tcc * 128).astype(ml_dtypes.bfloat16)
        return out

    ohP = onehots(layP["dpart"])
    ohL = onehots(layL["dpart"])

    in_maps = []
    for c in range(NC):
        in_maps.append({
            "x_sh": x_s[c], "y_sh": y_s[c], "dw_sh": dw_s[c],
            "mask_sh": mk_s[c], "deg_sh": dg_s[c],
            "idxP_d": idxP128[c], "idxL_d": idxL128[c],
            "ohP_d": ohP[c], "ohL_d": ohL[c],
            "W0": np.asarray(W_gcn0, np.float32),
            "W1": np.asarray(W_gcn1, np.float32),
            "Wl": Wl_flat,
            "Wf": np.asarray(W_fuse, np.float32),
            "b_all": b_all,
        })
    res = bass_utils.run_bass_kernel_spmd(nc, in_maps, core_ids=list(range(NC)))
    out = np.empty((n_nodes, OUT), np.float32)
    for c in range(NC):
        sel = core_of == np.int64(c)
        out[sel] = res.results[c]["out_sh"][local_of[sel]]
    return out
